# revision 28
# baseline (speedup 1.0000x reference)
"""Dilated self-attention TRN2 Bass kernel.

Problem (hardcoded): B=2, N=8192, C=256, WS=[2048,4096,8192], RS=[1,2,4],
HEAD_IDX=0 -> G=7 groups of s=2048 rows each.

Sharding: 8 cores, core d = (b=d//4, q=d%4) owns output positions
[2048q, 2048(q+1)) of batch b.  Each core computes, fully locally:
  job A: the r=1 segment group g_q of its quarter (2048 queries, causal)
  job B: the 1024-query half of the r=2 group whose outputs land in its quarter
  job C: the 512-query quarter of the r=4 group landing in its quarter
Attention is computed unnormalized: U = exp(scores/16 - 2) @ [V | 1], so the
last column carries the softmax denominator.  The cross-group scatter-add
combine is local (placement-matrix matmuls + DVE adds into a resident SBUF
accumulator); then out = U[:, :256] / U[:, 256].

Precision: projections and scores run in f32r.  The probs@V stage runs in
fp8-e4m3 DoubleRow mode (0.5 cycles/row, 256-deep contraction): exp writes
e4m3 probs directly (global shift exp(s/16 - 2) keeps probs in fp8 range;
the shift cancels in U/denom), and V is split into an fp8 hi + fp8 residual
pair so V retains ~0.2% accuracy.  Probs k-tiles are paired into [128,2,512]
DoubleRow tiles; acausal heads of diagonal tiles are zeroed so every pair is
uniformly DR-consumable for every 128-query output block.
"""

import numpy as np

B, N, C = 2, 8192, 256
S = 2048          # rows per group
NCORES = 8
SCALE = 0.0625    # 1/sqrt(256)
SHIFT = -2.0      # exp(s*SCALE + SHIFT): keeps probs within fp8-e4m3 range
NEG = -1.0e9

_PROG = None      # cached compiled Bass program


def _build_program():
    import concourse.mybir as mybir
    import concourse.tile as tile
    from concourse import bacc

    F32 = mybir.dt.float32
    F32R = mybir.dt.float32r
    E4 = mybir.dt.float8e4
    DR = mybir.MatmulPerfMode.DoubleRow
    Exp = mybir.ActivationFunctionType.Exp
    Mult = mybir.AluOpType.mult
    Add = mybir.AluOpType.add

    nc = bacc.Bacc("TRN2", target_bir_lowering=False, debug=False,
                   num_devices=NCORES)

    xA = nc.dram_tensor("xA", [C, S], F32R, kind="ExternalInput")
    xB = nc.dram_tensor("xB", [C, S], F32R, kind="ExternalInput")
    xC = nc.dram_tensor("xC", [C, S], F32R, kind="ExternalInput")
    w_d = nc.dram_tensor("w", [C, 3 * C], F32R, kind="ExternalInput")
    bias_d = nc.dram_tensor("bias", [128, 21], F32, kind="ExternalInput")
    out_d = nc.dram_tensor("out", [S, C], F32, kind="ExternalOutput")

    # job specs: (x dram, n q rows, q row offset in slab, prefix k-tiles,
    #             bias dram col or None, scatter stride)
    # order A, C, B: A's queries live in slab wave 0 with no prefix k-tiles,
    # so PE starts scoring ~2us in while C/B slabs stream; acc adds commute,
    # so per-tile finalize happens in B (emitted last)
    jobs = [
        dict(x=xC, nq=512, q0=1536, P=12, bias=8, stride=4),
        dict(x=xB, nq=1024, q0=1024, P=8, bias=0, stride=2),
        dict(x=xA, nq=2048, q0=0, P=0, bias=None, stride=1),
    ]

    with tile.TileContext(nc) as tc:
        with (
            tc.tile_pool(name="const", bufs=1) as cpool,
            tc.tile_pool(name="xsb", bufs=2) as xpool,
            tc.tile_pool(name="ktp", bufs=4) as ktpool,
            tc.tile_pool(name="qtp", bufs=2) as qtpool,
            tc.tile_pool(name="vext", bufs=20) as vpool,
            tc.tile_pool(name="probs", bufs=36) as ppool,
            tc.tile_pool(name="stage", bufs=4) as spool,
            tc.tile_pool(name="acc", bufs=1) as apool,
            tc.tile_pool(name="fin", bufs=4) as fpool,
            tc.tile_pool(name="ps_s", bufs=3, space="PSUM") as ps_scores,
            tc.tile_pool(name="ps_u", bufs=2, space="PSUM") as ps_u,
            tc.tile_pool(name="ps_p", bufs=3, space="PSUM") as ps_proj,
        ):
            # ---- constants (weights first: every projection needs them) ----
            w_sb = {}   # (name, ci) -> AP [128, 256]
            wt = []
            for ci in range(2):
                t = cpool.tile([128, 3 * C], F32R, tag=f"wt{ci}", name=f"wt{ci}")
                eng = nc.sync if ci == 0 else nc.scalar
                eng.dma_start(t[:], w_d[128 * ci:128 * (ci + 1), :])
                wt.append(t)

            # ---- first job's first chunks (start PE asap) ----
            slab = {}   # (jn, ci) -> list of (tile, col offset)
            for ci in range(2):
                t = xpool.tile([128, 512], F32R, tag="xsb", name=f"xa0_{ci}")
                eng = nc.sync if ci == 0 else nc.scalar
                eng.dma_start(t[:], jobs[0]["x"][128 * ci:128 * (ci + 1), 0:512])
                slab[0, ci] = [(t, 0)]
            for i, nm in enumerate(("q", "k", "v")):
                for ci in range(2):
                    w_sb[nm, ci] = wt[ci][:, 256 * i:256 * (i + 1)]
            bias_t = cpool.tile([128, 21], F32, tag="bias")
            nc.sync.dma_start(bias_t[:], bias_d[:])

            # ---- remaining slab loads ----
            for ci in range(2):
                t = xpool.tile([128, 1536], F32R, tag="xsb1", name=f"xa1_{ci}")
                eng = nc.sync if ci == 0 else nc.scalar
                eng.dma_start(t[:], jobs[0]["x"][128 * ci:128 * (ci + 1), 512:2048])
                slab[0, ci].append((t, 512))
            for jn2 in (1, 2):
                xd = jobs[jn2]["x"]   # prefetch in job order
                for ci in range(2):
                    t = xpool.tile([128, 2048], F32R, bufs=4, tag="xsb2",
                                   name=f"x{jn2}_{ci}")
                    eng = nc.sync if ci == 0 else nc.scalar
                    eng.dma_start(t[:], xd[128 * ci:128 * (ci + 1), :])
                    slab[jn2, ci] = [(t, 0)]

            def xslice(jn2, ci, c0, c1):
                for t, off in slab[jn2, ci]:
                    if c0 >= off and c1 - off <= t.shape[-1]:
                        return t[:, c0 - off:c1 - off]
                raise AssertionError((jn2, ci, c0, c1))

            ones_t = cpool.tile([128, 128], F32, tag="ones")
            nc.gpsimd.memset(ones_t[:], 1.0)
            ones01 = cpool.tile([128, 2, 2], E4, tag="ones01")
            nc.vector.memset(ones01[:, :, 0:1], 1.0)
            nc.vector.memset(ones01[:, :, 1:2], 0.0)
            mtri_f = cpool.tile([128, 128], F32, tag="fscratch", name="mtri_f")
            nc.gpsimd.affine_select(
                out=mtri_f[:], in_=ones_t[:],
                compare_op=mybir.AluOpType.is_ge,
                fill=0.0, base=0,
                pattern=[[1, 128]], channel_multiplier=-1,
            )
            mtri = cpool.tile([128, 128], E4, tag="mtri")
            nc.vector.tensor_copy(mtri[:], mtri_f[:])

            # placement matrices: P[m, p] = 1 iff p == stride*m - 128*u
            # (Z = P.T @ U puts U row m onto partition stride*m - 128*u)
            pmats = {}
            for stride, u in [(2, 0), (2, 1), (4, 0), (4, 1), (4, 2), (4, 3)]:
                pf = cpool.tile([128, 128], F32, tag="fscratch",
                                name=f"pmf{stride}_{u}")
                nc.gpsimd.affine_select(
                    out=pf[:], in_=ones_t[:],
                    compare_op=mybir.AluOpType.is_equal,
                    fill=0.0, base=128 * u,
                    pattern=[[1, 128]], channel_multiplier=-stride,
                )
                pm = cpool.tile([128, 128], F32R, tag=f"pm{stride}_{u}",
                                name=f"pm{stride}_{u}")
                nc.vector.tensor_copy(pm[:], pf[:])
                pmats[stride, u] = pm

            # persistent accumulator: 16 tiles of [128, 257] covering the
            # 2048 output positions of this core's quarter
            acc = [apool.tile([128, 257], F32, tag=f"acc{t}", name=f"acc{t}")
                   for t in range(16)]
            for t in range(16):
                nc.gpsimd.memset(acc[t][:], 0.0)

            # ---- finalize setup: emitted per-tile as soon as the tile's
            # last accumulator add (job A) lands ----
            fin = [fpool.tile([128, 2, 256], F32, bufs=1, tag=f"fing{g}",
                              name=f"fing{g}")
                   for g in range(8)]
            out_r = out_d.rearrange("(g t p) c -> g p t c", p=128, t=2)

            def finalize_tile(t):
                g, ti = divmod(t, 2)
                rec = fpool.tile([128, 1], F32, tag="rec", name="rec")
                nc.vector.reciprocal(rec[:], acc[t][:, 256:257])
                nc.vector.tensor_scalar_mul(fin[g][:, ti, :],
                                            acc[t][:, 0:256], rec[:])
                if ti == 1:
                    eng = nc.sync if g % 2 == 0 else nc.scalar
                    eng.dma_start(out_r[g], fin[g][:])

            # deferred U-block emitter: lags one q-block behind the score
            # emission so the in-order PE queue always has score matmuls to
            # run while exp/mask of the previous block completes
            pending_u = []

            def flush_u(keep=0):
                while len(pending_u) > keep:
                    pending_u.pop(0)()

            # ---- jobs ----
            for jn, job in enumerate(jobs):
                nq, q0, P = job["nq"], job["q0"], job["P"]
                stride = job["stride"]
                nkt_all = 16          # k/v tiles per job (always full slab)

                # projections, emitted in chunk waves so PE can start
                # as soon as the first 512-column slab chunk lands
                kt_sb = [ktpool.tile([128, S], F32R, tag="kt", name=f"kt{jn}_{_i}")
                         for _i in range(2)]
                qt_sb = [qtpool.tile([128, nq], F32R, tag="qt", name=f"qt{jn}_{_i}")
                         for _i in range(2)]
                # v pairs: DR layout [128, 2, 258]; plane kj = k-tile 2t+kj;
                # col 256 = ones (denominator), col 257 = 0 pad
                vh = [None] * 8
                vl = [None] * 8
                for kc in range(4):
                    for co in range(2):
                        ps = ps_proj.tile([128, 512], F32, tag="proj")
                        for ci in range(2):
                            nc.tensor.matmul(
                                ps[:], w_sb["k", ci][:, 128 * co:128 * (co + 1)],
                                xslice(jn, ci, 512 * kc, 512 * (kc + 1)),
                                start=(ci == 0), stop=(ci == 1))
                        eng = nc.scalar if co == 0 else nc.vector
                        eng_copy = (nc.scalar.copy if co == 0
                                    else nc.vector.tensor_copy)
                        eng_copy(kt_sb[co][:, 512 * kc:512 * (kc + 1)], ps[:])
                    if 512 * kc >= q0:
                        qc = (512 * kc - q0) // 512
                        for co in range(2):
                            ps = ps_proj.tile([128, 512], F32, tag="proj")
                            for ci in range(2):
                                nc.tensor.matmul(
                                    ps[:], w_sb["q", ci][:, 128 * co:128 * (co + 1)],
                                    xslice(jn, ci, q0 + 512 * qc,
                                           q0 + 512 * (qc + 1)),
                                    start=(ci == 0), stop=(ci == 1))
                            nc.vector.tensor_copy(
                                qt_sb[co][:, 512 * qc:512 * (qc + 1)], ps[:])
                    for kt in range(4 * kc, 4 * kc + 4):
                        ps = ps_proj.tile([128, 256], F32, tag="proj", name="psv")
                        for ci in range(2):
                            nc.tensor.matmul(
                                ps[:], xslice(jn, ci, 128 * kt, 128 * (kt + 1)),
                                w_sb["v", ci][:],
                                start=(ci == 0), stop=(ci == 1))
                        t2, kj = divmod(kt, 2)
                        if kj == 0:
                            vh[t2] = vpool.tile([128, 2, 258], E4, tag="vh", name=f"vh{jn}_{t2}")
                            vl[t2] = vpool.tile([128, 2, 258], E4, tag="vl", name=f"vl{jn}_{t2}")
                        vf = vpool.tile([128, 256], F32, tag="vf", name="vf", bufs=6)
                        if kt % 2 == 0:
                            nc.scalar.copy(vf[:], ps[:])
                        else:
                            nc.vector.tensor_copy(vf[:], ps[:])
                        # fp8 split on Pool (SBUF-only): vh = q8(v),
                        # vl = (vh * -1) + v
                        nc.gpsimd.tensor_copy(vh[t2][:, kj, 0:256], vf[:])
                        nc.gpsimd.tensor_sub(
                            vl[t2][:, kj, 0:256], vf[:],
                            vh[t2][:, kj, 0:256])
                        if kj == 1:
                            nc.gpsimd.tensor_copy(vh[t2][:, :, 256:258],
                                                  ones01[:])
                            nc.gpsimd.memset(vl[t2][:, :, 256:258], 0.0)

                flush_u()   # previous job's last q block

                # ---- attention over 512-wide q blocks ----
                for i in range(nq // 512):
                    nkt = P + 4 * i + 4
                    # probs DR pairs: tile t covers k-tiles (2t, 2t+1)
                    probs = [ppool.tile([128, 2, 512], E4, tag="probs", name=f"pb{_t}")
                             for _t in range(nkt // 2)]
                    for kt in range(nkt):
                        jd = kt - (P + 4 * i)
                        # diag tiles only need q columns >= 128*jd; clamp to
                        # >=256 wide to keep the f32r full-rate path
                        qoff = 0 if jd < 0 else min(128 * jd, 256)
                        w = 512 - qoff
                        ps = ps_scores.tile([128, 512], F32, tag="scores")
                        for ci in range(2):
                            nc.tensor.matmul(
                                ps[:, 0:w], kt_sb[ci][:, 128 * kt:128 * (kt + 1)],
                                qt_sb[ci][:, 512 * i + qoff:512 * (i + 1)],
                                start=(ci == 0), stop=(ci == 1))
                        t2, kj = divmod(kt, 2)
                        pb = probs[t2]
                        if kt < P:
                            b0 = job["bias"]
                            bias_ap = bias_t[:, b0 + kt:b0 + kt + 1]
                        else:
                            bias_ap = bias_t[:, 20:21]
                        nc.scalar.activation(pb[:, kj, qoff:512], ps[:, 0:w],
                                             Exp, bias=bias_ap, scale=SCALE)
                        if jd >= 0:
                            # causal mask: triangle on the diagonal 128 block,
                            # then zero the acausal head [0, 128*jd)
                            c0 = 128 * jd
                            nc.vector.tensor_mul(
                                pb[:, kj, c0:c0 + 128],
                                pb[:, kj, c0:c0 + 128], mtri[:])
                            if jd >= 1:
                                nc.gpsimd.memset(pb[:, kj, 0:c0], 0.0)

                    def emit_u(i=i, nkt=nkt, probs=probs, vh=vh, vl=vl,
                               stride=stride):
                        for j in range(4):
                            ups = ps_u.tile([128, 258], F32, tag="u", name="ups")
                            npair = nkt // 2
                            for t2 in range(npair):
                                pb_sl = probs[t2][:, 0:2, 128 * j:128 * (j + 1)]
                                nc.tensor.matmul(ups[:], pb_sl, vh[t2][:],
                                                 start=(t2 == 0), stop=False,
                                                 perf_mode=DR)
                                nc.tensor.matmul(ups[:], pb_sl, vl[t2][:],
                                                 start=False,
                                                 stop=(t2 == npair - 1),
                                                 perf_mode=DR)
                            t_local = 4 * i + j  # q tile index within job
                            if stride == 1:
                                nc.vector.tensor_add(acc[t_local][:],
                                                     acc[t_local][:],
                                                     ups[:, 0:257])
                                finalize_tile(t_local)
                            else:
                                st = spool.tile([128, 258], F32R, tag="stage")
                                with tc.high_priority(offset=20):
                                    nc.vector.tensor_copy(st[:], ups[:])
                                with tc.high_priority(offset=-40):
                                    for u in range(stride):
                                        zps = ps_u.tile([128, 258], F32,
                                                        tag="u", name="zps")
                                        nc.tensor.matmul(zps[:],
                                                         pmats[stride, u][:],
                                                         st[:], start=True,
                                                         stop=True)
                                        at = acc[stride * t_local + u]
                                        nc.vector.tensor_add(at[:], at[:],
                                                             zps[:, 0:257])

                    flush_u(keep=3)
                    pending_u.append(emit_u)

            flush_u()   # job A's last q block

    nc.compile()
    return nc


def _get_program():
    global _PROG
    if _PROG is None:
        _PROG = _build_program()
    return _PROG


def make_in_maps(x, Wq, Wk, Wv):
    """Host-side sharding: pure gather / transpose / zero-pad, no arithmetic."""
    x = np.asarray(x, dtype=np.float32)
    Wq = np.ascontiguousarray(np.asarray(Wq, dtype=np.float32))
    Wk = np.ascontiguousarray(np.asarray(Wk, dtype=np.float32))
    Wv = np.ascontiguousarray(np.asarray(Wv, dtype=np.float32))
    in_maps = []
    for d in range(NCORES):
        b, q = divmod(d, 4)
        xA = np.ascontiguousarray(x[b, 2048 * q:2048 * (q + 1), :].T)

        seg = 0 if q < 2 else 4096
        grp2 = x[b, seg:seg + 4096:2, :]          # [2048, 256]
        r0 = 1024 * (q % 2)
        if r0 == 1024:
            rowsB = grp2                           # prefix real + diag
        else:
            rowsB = np.concatenate(
                [np.zeros((1024, C), np.float32), grp2[0:1024]], axis=0)
        xB = np.ascontiguousarray(rowsB.T)

        grp4 = x[b, 0:8192:4, :]                  # [2048, 256]
        r0c = 512 * q
        rowsC = np.concatenate(
            [grp4[0:r0c], np.zeros((1536 - r0c, C), np.float32),
             grp4[r0c:r0c + 512]], axis=0)
        xC = np.ascontiguousarray(rowsC.T)

        # exp bias: SHIFT for live k-tiles, NEG (masks to 0) for padded ones
        bias = np.full((128, 21), SHIFT, np.float32)
        if r0 != 1024:
            bias[:, 0:8] = NEG
        bias[:, 8 + 4 * q:20] = NEG

        in_maps.append({
            "xA": xA, "xB": xB, "xC": xC,
            "w": np.ascontiguousarray(np.concatenate([Wq, Wk, Wv], axis=1)),
            "bias": bias,
        })
    return in_maps


def kernel(x, Wq, Wk, Wv):
    from concourse.bass_utils import run_bass_kernel_spmd

    nc = _get_program()
    in_maps = make_in_maps(x, Wq, Wk, Wv)
    res = run_bass_kernel_spmd(nc, in_maps, core_ids=list(range(NCORES)))
    out = np.empty((B, N, C), np.float32)
    for d in range(NCORES):
        b, q = divmod(d, 4)
        out[b, 2048 * q:2048 * (q + 1), :] = res.results[d]["out"]
    return out
